# revision 6
# baseline (speedup 1.0000x reference)
"""Trainium2 Bass kernel for the BrushStroke renderer.

out[b,c,h,w] = (1/N) * sum_n sum_{p,q} Fy[b,n,h,p] * patches[b,n,c,p,q] * Fx[b,n,w,q]

with Fx/Fy normalized Gaussian filter banks (sigma=0.2) over a padded 272-wide
coordinate grid.

Strategy (8 NeuronCores, data-parallel over batch B=32 -> 4 batches/core):
  - strokes grouped into 8 chunks of 8; SBUF partition dim = (stroke-in-chunk,
    patch-dim) = 8*16 = 128.
  - Gaussian filters: d = coords - mu on DVE, d^2 on DVE, exp(-12.5 d^2) on
    ScalarE with fused accum_out row-sums (free normalization sums).
  - Stage 1 (contract p): block-diagonal patch matrix (host packed, fp16) as
    the stationary matmul operand -> U^T[(n,q), h] in PSUM, K=128-dense.
  - Stage 2 (contract n,q): out[h, w] += U^T.T @ FxN accumulated over chunks
    in PSUM; output lands directly in [h, w] layout.
"""

import sys

import numpy as np

_B, _N, _C, _PS = 32, 64, 3, 16
_IMG, _PAD, _GRID = 256, 8, 272
_NCORES = 8
_BLOC = _B // _NCORES      # batches per core
_NCHUNK = 8                # stroke chunks
_CPB = _N // _NCHUNK       # strokes per chunk (8)
_CSCALE = 16.0             # coordinate downscale so d^2 fits fp16
_EXP_SCALE = -12.5 * _CSCALE * _CSCALE  # -1/(2*sigma^2) * CSCALE^2
_EPS = 1e-7

_cache = {}


def _build_nc():
    if "nc" in _cache:
        return _cache["nc"]
    sys.path.insert(0, "/opt/trn_rl_repo")
    import concourse.tile as tile
    from concourse import bacc, mybir
    from contextlib import ExitStack

    fp32 = mybir.dt.float32
    fp16 = mybir.dt.float16
    AF = mybir.ActivationFunctionType
    OP = mybir.AluOpType

    nc = bacc.Bacc(
        "TRN2", target_bir_lowering=False, debug=False, enable_asserts=False
    )

    bigp = nc.dram_tensor(
        "bigp", [_BLOC, _C, 128, _NCHUNK * 128], fp16, kind="ExternalInput"
    ).ap()
    grepl = nc.dram_tensor(
        "grepl", [_BLOC, 128, 16], fp32, kind="ExternalInput"
    ).ap()
    cbc = nc.dram_tensor("cbc", [128, _GRID], fp32, kind="ExternalInput").ap()
    noffs = nc.dram_tensor("noffs", [128, 1], fp32, kind="ExternalInput").ap()
    outp = nc.dram_tensor(
        "outp", [_BLOC, _C, _IMG, _IMG], fp32, kind="ExternalOutput"
    ).ap()

    with tile.TileContext(nc) as tc, ExitStack() as ctx:
        cpool = ctx.enter_context(tc.tile_pool(name="const", bufs=1))
        gpool = ctx.enter_context(tc.tile_pool(name="g", bufs=2))
        mupool = ctx.enter_context(tc.tile_pool(name="mu", bufs=2))
        dpool = ctx.enter_context(tc.tile_pool(name="d", bufs=4))
        d2pool = ctx.enter_context(tc.tile_pool(name="d2", bufs=4))
        fpool = ctx.enter_context(tc.tile_pool(name="fraw", bufs=20))
        spool = ctx.enter_context(tc.tile_pool(name="sums", bufs=2))
        fnpool = ctx.enter_context(tc.tile_pool(name="fnorm", bufs=32))
        bppool = ctx.enter_context(tc.tile_pool(name="bp", bufs=3))
        uspool = ctx.enter_context(tc.tile_pool(name="us", bufs=3))
        obpool = ctx.enter_context(tc.tile_pool(name="ob", bufs=3))
        pupool = ctx.enter_context(tc.tile_pool(name="pu", bufs=2, space="PSUM"))
        popool = ctx.enter_context(tc.tile_pool(name="po", bufs=2, space="PSUM"))

        cb_t = cpool.tile([128, _GRID], fp32)
        nc.sync.dma_start(cb_t[:], cbc)
        no_t = cpool.tile([128, 1], fp32)
        nc.sync.dma_start(no_t[:], noffs)

        for b in range(_BLOC):
            g_t = gpool.tile([128, 16], fp32)
            nc.sync.dma_start(g_t[:], grepl[b])
            # negmu[:, ca] = (g * -256 - offs) / CSCALE  (per-partition AP)
            negmu = mupool.tile([128, 16], fp32)
            nc.vector.tensor_scalar(
                negmu[:], g_t[:], -256.0 / _CSCALE, no_t[:], OP.mult, OP.add
            )

            sums = spool.tile([128, 16], fp32)
            f_raw = []
            for ca in range(16):
                d = dpool.tile([128, _GRID], fp16)
                nc.any.tensor_scalar(
                    d[:], cb_t[:], negmu[:, ca : ca + 1], None, OP.add
                )
                d2 = d2pool.tile([128, _GRID], fp16)
                nc.any.tensor_tensor(d2[:], d[:], d[:], OP.mult)
                f = fpool.tile([128, _GRID], fp16)
                nc.scalar.activation(
                    f[:],
                    d2[:],
                    AF.Exp,
                    scale=_EXP_SCALE,
                    accum_out=sums[:, ca : ca + 1],
                )
                f_raw.append(f)

            s2 = spool.tile([128, 16], fp32, tag="s2")
            nc.vector.tensor_scalar_add(s2[:], sums[:], _EPS)
            sinv = spool.tile([128, 16], fp32, tag="sinv")
            nc.vector.reciprocal(sinv[:], s2[:])

            fn = []
            for ca in range(16):
                t = fnpool.tile([128, _IMG], fp16)
                src = f_raw[ca][:, _PAD : _PAD + _IMG]
                if ca < 8:  # x-axis filters (consumed by stage 2)
                    nc.any.tensor_scalar(
                        t[:], src, sinv[:, ca : ca + 1], None, OP.mult
                    )
                else:  # y-axis filters: fold in the 1/N stroke mean
                    nc.any.tensor_scalar(
                        t[:],
                        src,
                        sinv[:, ca : ca + 1],
                        1.0 / _N,
                        OP.mult,
                        OP.mult,
                    )
                fn.append(t)

            for c in range(_C):
                bp = bppool.tile([128, _NCHUNK * 128], fp16)
                nc.sync.dma_start(bp[:], bigp[b, c])

                us_g = []
                for g in range(2):
                    pu = pupool.tile([128, 4 * _IMG], fp32)
                    for kk in range(4):
                        k = 4 * g + kk
                        # U^T[(i,q), h] = sum_(i,p) BigP_k[(i,p),(i,q)] FyN_k[(i,p), h]
                        nc.tensor.matmul(
                            pu[:, kk * _IMG : (kk + 1) * _IMG],
                            bp[:, k * 128 : (k + 1) * 128],
                            fn[8 + k][:],
                            start=True,
                            stop=True,
                        )
                    us = uspool.tile([128, 4 * _IMG], fp16)
                    nc.any.tensor_copy(us[:], pu[:])
                    us_g.append(us)
                # one PSUM bank (tile) per output h-half; accumulation groups
                # must not interleave within a bank (bank-level start/stop)
                for hh in range(2):
                    po = popool.tile([128, _IMG], fp32, tag=f"po{hh}")
                    for g in range(2):
                        for kk in range(4):
                            k = 4 * g + kk
                            # out[h, w] += U^T.T @ FxN_k
                            nc.tensor.matmul(
                                po[:],
                                us_g[g][
                                    :,
                                    kk * _IMG + hh * 128 : kk * _IMG + hh * 128 + 128,
                                ],
                                fn[k][:],
                                start=(k == 0),
                                stop=(k == 7),
                            )
                    ob = obpool.tile([128, _IMG], fp32)
                    nc.any.tensor_copy(ob[:], po[:])
                    nc.sync.dma_start(
                        outp[b, c, hh * 128 : (hh + 1) * 128, :], ob[:]
                    )

    nc.compile()
    _cache["nc"] = nc
    return nc


def _host_pack(brushes: np.ndarray, patches: np.ndarray):
    """Shard + repack inputs for the 8 cores (layout only, no math)."""
    brushes = np.asarray(brushes, np.float32)
    patches = np.asarray(patches, np.float32)

    # coordinate grid constants (shared by all cores), pre-divided by CSCALE
    cbc = np.broadcast_to(
        ((np.arange(_GRID, dtype=np.float32) - _PAD) / _CSCALE)[None, :],
        (128, _GRID),
    ).copy()
    noffs = (
        (7.5 - (np.arange(128, dtype=np.float32) % 16)) / _CSCALE
    ).reshape(128, 1)

    in_maps = []
    for cid in range(_NCORES):
        bs = brushes[cid * _BLOC : (cid + 1) * _BLOC]  # [4, 64, 2]
        ps = patches[cid * _BLOC : (cid + 1) * _BLOC]  # [4, 64, 3, 16, 16]

        # block-diagonal patch operand: A[b, c, 16i+p, k*128 + 16i+q]
        A = np.zeros((_BLOC, _C, 128, _NCHUNK, 128), np.float16)
        P6 = ps.reshape(_BLOC, _NCHUNK, _CPB, _C, _PS, _PS)  # [b,k,i,c,p,q]
        for i in range(_CPB):
            A[:, :, 16 * i : 16 * (i + 1), :, 16 * i : 16 * (i + 1)] = P6[
                :, :, i
            ].transpose(0, 2, 3, 1, 4)
        A = A.reshape(_BLOC, _C, 128, _NCHUNK * 128)

        # per-partition stroke centers: grepl[b, 16i+r, axis*8+k] = brushes[b, 8k+i, axis]
        G = bs.reshape(_BLOC, _NCHUNK, _CPB, 2)  # [b, k, i, axis]
        G = G.transpose(0, 2, 3, 1).reshape(_BLOC, _CPB, 16)  # [b, i, (axis,k)]
        grepl = np.repeat(G, 16, axis=1).astype(np.float32)  # [b, 128, 16]

        in_maps.append(
            {
                "bigp": A,
                "grepl": grepl,
                "cbc": cbc,
                "noffs": noffs,
            }
        )
    return in_maps


def _run(brushes, patches, trace=False):
    nc = _build_nc()
    sys.path.insert(0, "/opt/trn_rl_repo")
    from concourse import bass_utils

    in_maps = _host_pack(brushes, patches)
    res = bass_utils.run_bass_kernel_spmd(
        nc, in_maps, core_ids=list(range(_NCORES)), trace=trace
    )
    outs = [res.results[cid]["outp"] for cid in range(_NCORES)]
    full = np.concatenate(outs, axis=0).astype(np.float32)  # [32, 3, 256, 256]
    return full, res


def kernel(brushes: np.ndarray, patches: np.ndarray) -> np.ndarray:
    out, _ = _run(brushes, patches, trace=False)
    return out


# revision 9
# speedup vs baseline: 7.5767x; 7.5767x over previous
"""Trainium2 Bass kernel for the BrushStroke renderer.

out[b,c,h,w] = (1/N) * sum_n sum_{p,q} Fy[b,n,h,p] * patches[b,n,c,p,q] * Fx[b,n,w,q]

with Fx/Fy normalized Gaussian filter banks (sigma=0.2) over a padded 272-wide
coordinate grid.

Strategy (8 NeuronCores, data-parallel over batch B=32 -> 4 batches/core):
  - strokes grouped into 8 chunks of 8; SBUF partition dim = (stroke-in-chunk,
    patch-dim) = 8*16 = 128.
  - Gaussian filters: d = coords - mu on DVE, d^2 on DVE, exp(-12.5 d^2) on
    ScalarE with fused accum_out row-sums (free normalization sums).
  - Stage 1 (contract p): block-diagonal patch matrix (host packed, fp16) as
    the stationary matmul operand -> U^T[(n,q), h] in PSUM, K=128-dense.
  - Stage 2 (contract n,q): out[h, w] += U^T.T @ FxN accumulated over chunks
    in PSUM; output lands directly in [h, w] layout.
"""

import sys

import numpy as np

_B, _N, _C, _PS = 32, 64, 3, 16
_IMG, _PAD, _GRID = 256, 8, 272
_NCORES = 8
_BLOC = _B // _NCORES      # batches per core
_NCHUNK = 8                # stroke chunks
_CPB = _N // _NCHUNK       # strokes per chunk (8)
_CSCALE = 16.0             # coordinate downscale so d^2 fits fp16
_EXP_SCALE = -12.5 * _CSCALE * _CSCALE  # -1/(2*sigma^2) * CSCALE^2
_EPS = 1e-7

_cache = {}


def _build_nc(reps: int = 1):
    if ("nc", reps) in _cache:
        return _cache[("nc", reps)]
    sys.path.insert(0, "/opt/trn_rl_repo")
    import concourse.tile as tile
    from concourse import bacc, mybir
    from contextlib import ExitStack

    fp32 = mybir.dt.float32
    fp16 = mybir.dt.float16
    AF = mybir.ActivationFunctionType
    OP = mybir.AluOpType

    nc = bacc.Bacc(
        "TRN2", target_bir_lowering=False, debug=False, enable_asserts=False
    )

    bigp = nc.dram_tensor(
        "bigp", [_BLOC, _C, 128, _NCHUNK * 128], fp16, kind="ExternalInput"
    ).ap()
    grepl = nc.dram_tensor(
        "grepl", [_BLOC, 128, 16], fp32, kind="ExternalInput"
    ).ap()
    cbc = nc.dram_tensor("cbc", [128, _GRID], fp32, kind="ExternalInput").ap()
    noffs = nc.dram_tensor("noffs", [128, 1], fp32, kind="ExternalInput").ap()
    outp = nc.dram_tensor(
        "outp", [_BLOC, _C, _IMG, _IMG], fp32, kind="ExternalOutput"
    ).ap()

    with tile.TileContext(nc) as tc, ExitStack() as ctx:
        cpool = ctx.enter_context(tc.tile_pool(name="const", bufs=1))
        gpool = ctx.enter_context(tc.tile_pool(name="g", bufs=2))
        mupool = ctx.enter_context(tc.tile_pool(name="mu", bufs=2))
        dpool = ctx.enter_context(tc.tile_pool(name="d", bufs=4))
        d2pool = ctx.enter_context(tc.tile_pool(name="d2", bufs=4))
        fpool = ctx.enter_context(tc.tile_pool(name="fraw", bufs=20))
        spool = ctx.enter_context(tc.tile_pool(name="sums", bufs=2))
        fnpool = ctx.enter_context(tc.tile_pool(name="fnorm", bufs=32))
        bppool = ctx.enter_context(tc.tile_pool(name="bp", bufs=3))
        uspool = ctx.enter_context(tc.tile_pool(name="us", bufs=3))
        obpool = ctx.enter_context(tc.tile_pool(name="ob", bufs=3))
        pupool = ctx.enter_context(tc.tile_pool(name="pu", bufs=2, space="PSUM"))
        popool = ctx.enter_context(tc.tile_pool(name="po", bufs=2, space="PSUM"))

        cb_t = cpool.tile([128, _GRID], fp32)
        nc.sync.dma_start(cb_t[:], cbc)
        no_t = cpool.tile([128, 1], fp32)
        nc.sync.dma_start(no_t[:], noffs)

        for _rep in range(reps):
          for b in range(_BLOC):
            g_t = gpool.tile([128, 16], fp32)
            nc.sync.dma_start(g_t[:], grepl[b])
            # negmu[:, ca] = (g * -256 - offs) / CSCALE  (per-partition AP)
            negmu = mupool.tile([128, 16], fp32)
            nc.vector.tensor_scalar(
                negmu[:], g_t[:], -256.0 / _CSCALE, no_t[:], OP.mult, OP.add
            )

            sums = spool.tile([128, 16], fp32)
            f_raw = []
            for ca in range(16):
                d = dpool.tile([128, _GRID], fp16)
                nc.any.tensor_scalar(
                    d[:], cb_t[:], negmu[:, ca : ca + 1], None, OP.add
                )
                d2 = d2pool.tile([128, _GRID], fp16)
                nc.any.tensor_tensor(d2[:], d[:], d[:], OP.mult)
                f = fpool.tile([128, _GRID], fp16)
                nc.scalar.activation(
                    f[:],
                    d2[:],
                    AF.Exp,
                    scale=_EXP_SCALE,
                    accum_out=sums[:, ca : ca + 1],
                )
                f_raw.append(f)

            s2 = spool.tile([128, 16], fp32, tag="s2")
            nc.vector.tensor_scalar_add(s2[:], sums[:], _EPS)
            sinv = spool.tile([128, 16], fp32, tag="sinv")
            nc.vector.reciprocal(sinv[:], s2[:])

            fn = []
            for ca in range(16):
                t = fnpool.tile([128, _IMG], fp16)
                src = f_raw[ca][:, _PAD : _PAD + _IMG]
                if ca < 8:  # x-axis filters (consumed by stage 2)
                    nc.any.tensor_scalar(
                        t[:], src, sinv[:, ca : ca + 1], None, OP.mult
                    )
                else:  # y-axis filters: fold in the 1/N stroke mean
                    nc.any.tensor_scalar(
                        t[:],
                        src,
                        sinv[:, ca : ca + 1],
                        1.0 / _N,
                        OP.mult,
                        OP.mult,
                    )
                fn.append(t)

            for c in range(_C):
                bp = bppool.tile([128, _NCHUNK * 128], fp16)
                nc.sync.dma_start(bp[:], bigp[b, c])

                us_g = []
                for g in range(2):
                    pu = pupool.tile([128, 4 * _IMG], fp32)
                    for kk in range(4):
                        k = 4 * g + kk
                        # U^T[(i,q), h] = sum_(i,p) BigP_k[(i,p),(i,q)] FyN_k[(i,p), h]
                        nc.tensor.matmul(
                            pu[:, kk * _IMG : (kk + 1) * _IMG],
                            bp[:, k * 128 : (k + 1) * 128],
                            fn[8 + k][:],
                            start=True,
                            stop=True,
                        )
                    us = uspool.tile([128, 4 * _IMG], fp16)
                    nc.any.tensor_copy(us[:], pu[:])
                    us_g.append(us)
                # one PSUM bank (tile) per output h-half; accumulation groups
                # must not interleave within a bank (bank-level start/stop)
                for hh in range(2):
                    po = popool.tile([128, _IMG], fp32, tag=f"po{hh}")
                    for g in range(2):
                        for kk in range(4):
                            k = 4 * g + kk
                            # out[h, w] += U^T.T @ FxN_k
                            nc.tensor.matmul(
                                po[:],
                                us_g[g][
                                    :,
                                    kk * _IMG + hh * 128 : kk * _IMG + hh * 128 + 128,
                                ],
                                fn[k][:],
                                start=(k == 0),
                                stop=(k == 7),
                            )
                    ob = obpool.tile([128, _IMG], fp32)
                    nc.any.tensor_copy(ob[:], po[:])
                    nc.sync.dma_start(
                        outp[b, c, hh * 128 : (hh + 1) * 128, :], ob[:]
                    )

    nc.compile()
    _cache[("nc", reps)] = nc
    return nc


def _host_pack(brushes: np.ndarray, patches: np.ndarray):
    """Shard + repack inputs for the 8 cores (layout only, no math)."""
    brushes = np.asarray(brushes, np.float32)
    patches = np.asarray(patches, np.float32)

    # coordinate grid constants (shared by all cores), pre-divided by CSCALE
    cbc = np.broadcast_to(
        ((np.arange(_GRID, dtype=np.float32) - _PAD) / _CSCALE)[None, :],
        (128, _GRID),
    ).copy()
    noffs = (
        (7.5 - (np.arange(128, dtype=np.float32) % 16)) / _CSCALE
    ).reshape(128, 1)

    in_maps = []
    for cid in range(_NCORES):
        bs = brushes[cid * _BLOC : (cid + 1) * _BLOC]  # [4, 64, 2]
        ps = patches[cid * _BLOC : (cid + 1) * _BLOC]  # [4, 64, 3, 16, 16]

        # block-diagonal patch operand: A[b, c, 16i+p, k*128 + 16i+q]
        A = np.zeros((_BLOC, _C, 128, _NCHUNK, 128), np.float16)
        P6 = ps.reshape(_BLOC, _NCHUNK, _CPB, _C, _PS, _PS)  # [b,k,i,c,p,q]
        for i in range(_CPB):
            A[:, :, 16 * i : 16 * (i + 1), :, 16 * i : 16 * (i + 1)] = P6[
                :, :, i
            ].transpose(0, 2, 3, 1, 4)
        A = A.reshape(_BLOC, _C, 128, _NCHUNK * 128)

        # per-partition stroke centers: grepl[b, 16i+r, axis*8+k] = brushes[b, 8k+i, axis]
        G = bs.reshape(_BLOC, _NCHUNK, _CPB, 2)  # [b, k, i, axis]
        G = G.transpose(0, 2, 3, 1).reshape(_BLOC, _CPB, 16)  # [b, i, (axis,k)]
        grepl = np.repeat(G, 16, axis=1).astype(np.float32)  # [b, 128, 16]

        in_maps.append(
            {
                "bigp": A,
                "grepl": grepl,
                "cbc": cbc,
                "noffs": noffs,
            }
        )
    return in_maps


def _run(brushes, patches, trace=False):
    nc = _build_nc()
    sys.path.insert(0, "/opt/trn_rl_repo")
    from concourse import bass_utils

    in_maps = _host_pack(brushes, patches)
    res = bass_utils.run_bass_kernel_spmd(
        nc, in_maps, core_ids=list(range(_NCORES)), trace=trace
    )
    outs = [res.results[cid]["outp"] for cid in range(_NCORES)]
    full = np.concatenate(outs, axis=0).astype(np.float32)  # [32, 3, 256, 256]
    return full, res


def kernel(brushes: np.ndarray, patches: np.ndarray) -> np.ndarray:
    out, _ = _run(brushes, patches, trace=False)
    return out
